# revision 47
# baseline (speedup 1.0000x reference)
"""Multi-head attention (B=2, S=2048, D=2048, H=16 causal) on 8 TRN2 cores.

Sharding: core c handles batch b = c//4 and head group g = c%4 (4 heads,
512 of the 2048 model dims). Tensor-parallel: q/k/v_proj rows (output
dims) are split by head group; o_proj columns (input dims) likewise, so
each core produces a partial [S, D] output that the host sums per batch.

Host prep per core (numpy):
  xt  = x[b].T              [D, S]   bf16   (d on partitions for matmul)
  wqt = q_proj[gslice].T    [D, 512] bf16
  wkt = k_proj[gslice].T    [D, 512] bf16
  wvt = v_proj[gslice].T    [D, 512] bf16
  wot = o_proj[:, gslice].T [512, D] bf16
Device phases (all matmuls bf16 with f32 PSUM accumulation):
  1. QT/KT [128, 4h, S] and V [128, 16st, 512] projections.
  2. Per (head, q-block of 512): scoresT[k, q] = KT.T @ QT, exp (no
     max-subtraction: |scores| <= ~10 for this distribution), causal
     mask via multiplicative shifted-triangular bf16 masks, then
     attnout.T[dv, q] = sum_k V[k, dv] * expT[k, q] and the softmax
     denominator via an all-ones [128, 1] stationary matmul; normalize
     with reciprocal * broadcast.
  3. out_partial[s, :] = attnoutT.T @ wot, f32 to DRAM.
"""

import math
import sys
import types

import numpy as np
import ml_dtypes

# If BASS_TRACE is set in the environment, run_bass_kernel_spmd imports
# antenv.axon_hooks, which not every image ships. Register a no-op stub so
# that path degrades to "hook isn't registered" instead of crashing.
try:
    import antenv.axon_hooks  # noqa: F401
except Exception:
    try:
        import antenv

        _stub = types.ModuleType("antenv.axon_hooks")
        _stub._hook = None
        _stub.set_axon_ntff_profile_hook = lambda h: setattr(_stub, "_hook", h)
        _stub.get_axon_ntff_profile_hook = lambda: _stub._hook
        sys.modules["antenv.axon_hooks"] = _stub
        antenv.axon_hooks = _stub
    except Exception:
        pass

import concourse.bass as bass
import concourse.tile as tile
import concourse.mybir as mybir
from concourse import library_config
from concourse.bass_utils import run_bass_kernel_spmd
from concourse.library_overlay import lower_extended_insts
from concourse.vector_clock import ScopedClock

D = 2048
S = 2048
GM = 512  # model dims per core (4 heads x 128)
NH = 4  # heads per core
DK = 128
DC = D // 128  # 16 contraction chunks
NQB = S // 512  # 4 q blocks
SCALE = 1.0 / math.sqrt(DK)
N_CORES = 8

BF16 = mybir.dt.bfloat16
F32 = mybir.dt.float32


def _patched_drain_and_barrier(self, tick_clock, wait_clock):
    # Walrus rejects a Drain carrying >2 sync waits ("Too many sync wait
    # commands"). Put the global-clock waits on standalone single-wait
    # EventSemaphore instructions ahead of the drain instead.
    nc = self.nc
    probe = nc.sync.nop(nofuse=True)
    wait_clock.add_sem_waits(probe.ins, ScopedClock({None: tick_clock.global_clock}))
    si = probe.ins.sync_info
    waits = list(si.on_wait) if si is not None else []
    if len(waits) > 1:
        probe.ins.sync_info = mybir.SyncInfo(
            on_wait=[waits[0]], on_update=list(si.on_update)
        )
        sems = {}
        for h in self.sems.allocated().values():
            sems[h.name] = h
            sems[h.num] = h
        for w in waits[1:]:
            assert w.wait_mode == "sem-ge-imm", w
            h = sems.get(w.ant_name) or sems.get(w.id)
            nc.sync.wait_ge(h, w.wait_value)
    nc.sync.drain()
    nc.all_engine_barrier()
    popped = nc._tile_sem_poison_stack.pop()
    assert popped is self._sem_poison
    nc.clear_and_free_semaphores(list(self.sems.allocated().values()))
    nc.all_engine_barrier()


tile.TileContext._drain_and_barrier = _patched_drain_and_barrier

def _dedup_ldweights(nc):
    """Drop an InstLdweights whose weights AP is identical to the previous
    one on the same basic block with only Matmult/EventSemaphore between —
    the stationary operand is still resident in the PE array, so the reload
    is pure overhead (~107ns serialized behind each matmul). Loops are
    ordered so 4 consecutive matmuls share a stationary."""
    keep_types = {"InstMatmult", "InstEventSemaphore"}
    n_drop = 0
    for fn in nc.m.functions:
        for bb in fn.blocks:
            out = []
            last_key = None
            for inst in bb.instructions:
                tname = type(inst).__name__
                if tname == "InstLdweights":
                    si = inst.sync_info
                    key = (str(inst.ins[0]), getattr(inst, "tile_position", None))
                    if last_key == key and not (si and si.on_update):
                        if si and si.on_wait:
                            ev = mybir.InstEventSemaphore(
                                name=nc.get_next_instruction_name(),
                                engine=inst.engine,
                                ins=[],
                                outs=[],
                                sync_info=mybir.SyncInfo(
                                    on_wait=list(si.on_wait), on_update=[]
                                ),
                            )
                            nc.register_instruction(ev)
                            out.append(ev)
                        del nc.inst_map[inst.name]
                        n_drop += 1
                        continue
                    last_key = key
                elif tname not in keep_types and str(inst.engine) == "EngineType.PE":
                    last_key = None
                out.append(inst)
            bb.instructions[:] = out
    return n_drop


def _split_excess_waits(nc, max_waits=1):
    """Walrus rejects instructions carrying more than a couple of sync wait
    commands. Move excess waits onto standalone EventSemaphore instructions
    inserted just before the offender on the same engine (same-queue program
    order makes this equivalent)."""
    for fn in nc.m.functions:
        for bb in fn.blocks:
            out = []
            for inst in bb.instructions:
                si = inst.sync_info
                if si is not None and len(si.on_wait) > max_waits:
                    waits = list(si.on_wait)
                    for w in waits[:-max_waits]:
                        ev = mybir.InstEventSemaphore(
                            name=nc.get_next_instruction_name(),
                            engine=inst.engine,
                            ins=[],
                            outs=[],
                            sync_info=mybir.SyncInfo(on_wait=[w], on_update=[]),
                        )
                        nc.register_instruction(ev)
                        out.append(ev)
                    inst.sync_info = mybir.SyncInfo(
                        on_wait=waits[-max_waits:], on_update=list(si.on_update)
                    )
                out.append(inst)
            bb.instructions[:] = out


def build_bass():
    nc = bass.Bass("TRN2", target_bir_lowering=False, debug=False, num_devices=N_CORES)

    xt_d = nc.declare_dram_parameter("xt", [D, S], BF16, isOutput=False)
    wqt_d = nc.declare_dram_parameter("wqt", [D, GM], BF16, isOutput=False)
    wkt_d = nc.declare_dram_parameter("wkt", [D, GM], BF16, isOutput=False)
    wvt_d = nc.declare_dram_parameter("wvt", [D, GM], BF16, isOutput=False)
    wot_d = nc.declare_dram_parameter("wot", [GM, D], BF16, isOutput=False)
    masks_d = nc.declare_dram_parameter("masks", [128, NQB * 512], BF16, isOutput=False)
    ones_d = nc.declare_dram_parameter("ones", [128, 1], BF16, isOutput=False)
    out_d = nc.declare_dram_parameter("out", [S, D], F32, isOutput=True)

    with tile.TileContext(nc) as tc:
        with (
            tc.tile_pool(name="const", bufs=1) as const_pool,
            tc.tile_pool(name="qkv", bufs=1) as qkv_pool,
            tc.tile_pool(name="ao", bufs=1) as ao_pool,
        ):
            # GPSIMD runs partition_broadcast + tensor_tensor divide; the
            # proxy library has both. Load once.
            nc.gpsimd.load_library(library_config.proxy)
            # Constants from host: all-ones column for denominator matmuls,
            # and 4 shifted-triangular causal masks (variant r keeps q >= k + 128r).
            ones_sb = const_pool.tile([128, 1], BF16)
            nc.sync.dma_start(ones_sb[:], ones_d[:])
            masks_sb = const_pool.tile([128, NQB, 512], BF16)
            nc.sync.dma_start(masks_sb[:], masks_d[:].rearrange("p (r q) -> p r q", r=NQB))

            qt_sb = qkv_pool.tile([128, NH, S], BF16)
            kt_sb = qkv_pool.tile([128, NH, S], BF16)
            v_sb = qkv_pool.tile([128, S // 128, GM], BF16)
            ao_sb = ao_pool.tile([128, NH, S], BF16)

            # ---------------- Phase 1: projections ----------------
            with (
                tc.tile_pool(name="xt", bufs=1) as xt_pool,
                tc.tile_pool(name="w", bufs=1) as w_pool,
            ):
                # DMA order matters: wq first, then xt chunk-by-chunk so the
                # first QT chain starts ~7us in and paces with chunk arrival
                # (each stall < HAM's 3.4us window), then wk/wv.
                xt_sb = xt_pool.tile([128, DC, S], BF16)
                w_tiles = {}
                for wname, wd in (("wq", wqt_d), ("wk", wkt_d), ("wv", wvt_d)):
                    w_tiles[wname] = w_pool.tile(
                        [128, DC, GM], BF16, tag=wname, name=f"w_{wname}"
                    )

                def _load_w(wname, wd):
                    for dc in range(DC):
                        nc.sync.dma_start(
                            w_tiles[wname][:, dc, :], wd[128 * dc : 128 * (dc + 1), :]
                        )

                for dc in range(DC):
                    if dc == 0:
                        # finest-grained first chunk: the very first matmul
                        # needs only wq[0] + xt[0][:, 0:512]
                        nc.sync.dma_start(
                            w_tiles["wq"][:, 0, :], wqt_d[0:128, :]
                        )
                        for s4 in range(NQB):
                            nc.sync.dma_start(
                                xt_sb[:, 0, 512 * s4 : 512 * (s4 + 1)],
                                xt_d[0:128, 512 * s4 : 512 * (s4 + 1)],
                            )
                        continue
                    nc.sync.dma_start(
                        xt_sb[:, dc, :], xt_d[128 * dc : 128 * (dc + 1), :]
                    )
                    nc.sync.dma_start(
                        w_tiles["wq"][:, dc, :], wqt_d[128 * dc : 128 * (dc + 1), :]
                    )
                _load_w("wk", wkt_d)
                _load_w("wv", wvt_d)

                # QT / KT: out tile [m=128, s=512], contraction over d.
                # dc outer over PAIRS of m tiles = 8 concurrent psum chains
                # (all 8 banks): per xt chunk the PE has ~1.7us of work, which
                # matches the chunk DMA arrival rate, so the in-order PE queue
                # does not stall during the load ramp. LDWEIGHTS still
                # amortized 4x over the st4-minor matmuls.
                with tc.tile_pool(name="psum1", bufs=8, space="PSUM") as psum1:
                    for wname, out_sb in (("wq", qt_sb), ("wk", kt_sb)):
                        w_sb = w_tiles[wname]
                        for mtp in range(NH // 2):
                            pss = [
                                psum1.tile(
                                    [128, 512],
                                    F32,
                                    tag="ps1",
                                    name=f"ps1_{wname}_{mtp}_{i}",
                                )
                                for i in range(8)
                            ]
                            for dc in range(DC):
                                for j in (0, 1):
                                    mt = 2 * mtp + j
                                    for st4 in range(NQB):
                                        nc.tensor.matmul(
                                            pss[4 * j + st4][:],
                                            lhsT=w_sb[:, dc, 128 * mt : 128 * (mt + 1)],
                                            rhs=xt_sb[
                                                :, dc, 512 * st4 : 512 * (st4 + 1)
                                            ],
                                            start=(dc == 0),
                                            stop=(dc == DC - 1),
                                        )
                            for j in (0, 1):
                                for st4 in range(NQB):
                                    nc.vector.tensor_copy(
                                        out_sb[
                                            :, 2 * mtp + j, 512 * st4 : 512 * (st4 + 1)
                                        ],
                                        pss[4 * j + st4][:],
                                    )
                    # V: out tile [s=128, dv=512] — same pool/tag as QT/KT so
                    # there is no PSUM pool swap (and no PE stall) between.
                    w_sb = w_tiles["wv"]
                    for st in range(S // 128):
                        ps = psum1.tile([128, 512], F32, tag="ps1", name=f"ps1v_{st}")
                        for dc in range(DC):
                            nc.tensor.matmul(
                                ps[:],
                                lhsT=xt_sb[:, dc, 128 * st : 128 * (st + 1)],
                                rhs=w_sb[:, dc, :],
                                start=(dc == 0),
                                stop=(dc == DC - 1),
                            )
                        nc.vector.tensor_copy(v_sb[:, st, :], ps[:])

            # -------- Phase 2 + 3 pools (reuse phase-1 SBUF/PSUM space) -----
            with (
                tc.tile_pool(name="wot", bufs=1) as wot_pool,
                tc.tile_pool(name="exp", bufs=2) as exp_pool,
                tc.tile_pool(name="dpair", bufs=2) as dpair_pool,
                tc.tile_pool(name="small", bufs=4) as small_pool,
                tc.tile_pool(name="ostage", bufs=6) as out_pool,
            ):
                wot_sb = wot_pool.tile([128, NH, D], BF16)
                for c4 in range(NH):
                    nc.sync.dma_start(
                        wot_sb[:, c4, :], wot_d[128 * c4 : 128 * (c4 + 1), :]
                    )

                # ---------------- Phase 2: attention ----------------
                phase2_pools = tc.tile_pool(name="psum_s", bufs=2, space="PSUM")
                psum_s = phase2_pools.__enter__()
                psum_o_cm = tc.tile_pool(name="psum_o", bufs=2, space="PSUM")
                psum_o = psum_o_cm.__enter__()
                psum_d_cm = tc.tile_pool(name="psum_d", bufs=2, space="PSUM")
                psum_d = psum_d_cm.__enter__()
                # Software pipeline: group (h, qb)'s attnV/denominator
                # matmuls are interleaved between the NEXT group's scores
                # matmuls, so the PE never sits waiting for ACT's exp of the
                # current group (which would re-throttle the HAM clock).
                groups = [(h, qb) for h in range(NH) for qb in range(NQB)]

                def make_score_ops(h, qb):
                    # One op per PAIR of k tiles: two scores matmuls into the
                    # two banks of a [128, 1024] PSUM tile, one wide exp, then
                    # the causal mask multiplies, then a DVE pair-sum that
                    # halves the denominator matmul count.
                    nkt = 4 * qb + 4  # k tiles with any unmasked element
                    qsl = slice(512 * qb, 512 * (qb + 1))
                    eT = exp_pool.tile([128, nkt, 512], BF16, tag="eT")
                    dp = dpair_pool.tile(
                        [128, nkt // 2, 512], BF16, tag="dpair", name=f"dp_{h}_{qb}"
                    )

                    def score_pair(kt0):
                        # Diagonal k tiles (r = kt - 4qb >= 0) only have valid
                        # scores for q >= 128r in this block: trim the matmul
                        # to that range. The exp still covers the full pair
                        # tile (the trimmed region holds garbage that is never
                        # read: O/D matmuls are trimmed identically, and the
                        # pair-sum strip is zeroed below).
                        ps = psum_s.tile(
                            [128, 1024], F32, tag="ps_s", name=f"ps_s_{h}_{qb}_{kt0}"
                        )
                        for j in (0, 1):
                            kt = kt0 + j
                            r = kt - 4 * qb
                            qo = 128 * r if r > 0 else 0
                            nc.tensor.matmul(
                                ps[:, 512 * j + qo : 512 * (j + 1)],
                                lhsT=kt_sb[:, h, 128 * kt : 128 * (kt + 1)],
                                rhs=qt_sb[:, h, 512 * qb + qo : 512 * (qb + 1)],
                                start=True,
                                stop=True,
                            )
                        r1 = kt0 + 1 - 4 * qb
                        if r1 < 0:
                            # non-diagonal pair: one wide exp over both banks
                            nc.scalar.activation(
                                eT[:, kt0 : kt0 + 2, :].rearrange("p a b -> p (a b)"),
                                ps[:],
                                mybir.ActivationFunctionType.Exp,
                                scale=SCALE,
                            )
                        else:
                            # diagonal pair: exact-range exps (the trimmed
                            # PSUM regions were never written)
                            for j in (0, 1):
                                r = kt0 + j - 4 * qb
                                qo = 128 * r if r > 0 else 0
                                nc.scalar.activation(
                                    eT[:, kt0 + j, qo:512],
                                    ps[:, 512 * j + qo : 512 * (j + 1)],
                                    mybir.ActivationFunctionType.Exp,
                                    scale=SCALE,
                                )
                        for j in (0, 1):
                            r = kt0 + j - 4 * qb
                            if r >= 0:
                                # only the 128-wide diagonal strip can have
                                # masked elements; beyond it the mask is all 1
                                qo = 128 * r
                                qe = qo + 128
                                nc.vector.tensor_mul(
                                    eT[:, kt0 + j, qo:qe],
                                    eT[:, kt0 + j, qo:qe],
                                    masks_sb[:, r, qo:qe],
                                )
                        r1 = kt0 + 1 - 4 * qb
                        if r1 >= 1:
                            qo0 = 128 * (r1 - 1)
                            qo1 = 128 * r1
                            nc.vector.memset(eT[:, kt0 + 1, qo0:qo1], 0.0)
                            nc.vector.tensor_add(
                                dp[:, kt0 // 2, qo0:512],
                                eT[:, kt0, qo0:512],
                                eT[:, kt0 + 1, qo0:512],
                            )
                        else:
                            nc.vector.tensor_add(
                                dp[:, kt0 // 2, :], eT[:, kt0, :], eT[:, kt0 + 1, :]
                            )

                    return (
                        eT,
                        dp,
                        [lambda k=k: score_pair(2 * k) for k in range(nkt // 2)],
                    )

                def make_av_ops(h, qb, eT, dp):
                    nkt = 4 * qb + 4
                    qsl = slice(512 * qb, 512 * (qb + 1))
                    state = {}

                    def o_op(kt):
                        if kt == 0:
                            state["psO"] = psum_o.tile(
                                [128, 512], F32, tag="ps_o", name=f"psO_{h}_{qb}"
                            )
                        r = kt - 4 * qb
                        qo = 128 * r if r > 0 else 0
                        nc.tensor.matmul(
                            state["psO"][:, qo:512],
                            lhsT=v_sb[:, kt, 128 * h : 128 * (h + 1)],
                            rhs=eT[:, kt, qo:512],
                            start=(kt == 0),
                            stop=(kt == nkt - 1),
                        )

                    def d_op(p):
                        # Over the DVE pair-sums; emitted as one consecutive
                        # burst so the identical all-ones LDWEIGHTS dedupe
                        # down to a single load.
                        if p == 0:
                            state["psD"] = psum_d.tile(
                                [1, 512], F32, tag="ps_d", name=f"psD_{h}_{qb}"
                            )
                        r0 = 2 * p - 4 * qb
                        qo = 128 * r0 if r0 > 0 else 0
                        nc.tensor.matmul(
                            state["psD"][0:1, qo:512],
                            lhsT=ones_sb[:, :],
                            rhs=dp[:, p, qo:512],
                            start=(p == 0),
                            stop=(p == nkt // 2 - 1),
                        )

                    def finish():
                        # reciprocal_approx_fast (~18 bits, 5x faster than the
                        # ~6 cyc/elem exact DVE reciprocal; denominators are
                        # well in range), broadcast on GPSIMD, multiply on DVE.
                        den = small_pool.tile([1, 512], F32, tag="den", name=f"den_{h}_{qb}")
                        nc.vector.tensor_copy(den[:], state["psD"][:])
                        rcp = small_pool.tile([1, 512], F32, tag="rcp", name=f"rcp_{h}_{qb}")
                        nc.vector.reciprocal_approx_fast(rcp[:], den[:])
                        rcpb = small_pool.tile([128, 512], F32, tag="rcpb", name=f"rcpb_{h}_{qb}")
                        nc.gpsimd.partition_broadcast(rcpb[:], rcp[:])
                        nc.vector.tensor_mul(ao_sb[:, h, qsl], state["psO"][:], rcpb[:])

                    return (
                        [lambda kt=kt: o_op(kt) for kt in range(nkt)],
                        [lambda p=p: d_op(p) for p in range(nkt // 2)],
                        finish,
                    )

                pending_o, pending_d, pending_fin = [], [], None
                for h, qb in groups:
                    eT, dp, s_ops = make_score_ops(h, qb)
                    n_s, n_o = len(s_ops), len(pending_o)
                    emitted = 0
                    for i, s in enumerate(s_ops):
                        s()
                        want = (i + 1) * n_o // n_s
                        while emitted < want:
                            pending_o[emitted]()
                            emitted += 1
                    while emitted < n_o:
                        pending_o[emitted]()
                        emitted += 1
                    for op in pending_d:
                        op()
                    if pending_fin is not None:
                        pending_fin()
                    pending_o, pending_d, pending_fin = make_av_ops(h, qb, eT, dp)
                for op in pending_o:
                    op()
                for op in pending_d:
                    op()
                pending_fin()
                psum_d_cm.__exit__(None, None, None)
                psum_o_cm.__exit__(None, None, None)
                phase2_pools.__exit__(None, None, None)

                # ---------------- Phase 3: output projection ----------------
                # h inner, nt innermost: 4 psum chains share one stationary
                # ao chunk so LDWEIGHTS is amortized 4x.
                with tc.tile_pool(name="psum_3", bufs=5, space="PSUM") as psum_3:
                    for st in range(S // 128):
                        pss = [
                            psum_3.tile([128, 512], F32, tag="ps3", name=f"ps3_{st}_{n}")
                            for n in range(NQB)
                        ]
                        for h in range(NH):
                            for nt in range(NQB):
                                nc.tensor.matmul(
                                    pss[nt][:],
                                    lhsT=ao_sb[:, h, 128 * st : 128 * (st + 1)],
                                    rhs=wot_sb[:, h, 512 * nt : 512 * (nt + 1)],
                                    start=(h == 0),
                                    stop=(h == NH - 1),
                                )
                        for nt in range(NQB):
                            o_sb = out_pool.tile(
                                [128, 512], F32, tag="ost", name=f"ost_{st}_{nt}"
                            )
                            nc.vector.tensor_copy(o_sb[:], pss[nt][:])
                            nc.sync.dma_start(
                                out_d[
                                    128 * st : 128 * (st + 1),
                                    512 * nt : 512 * (nt + 1),
                                ],
                                o_sb[:],
                            )
    _dedup_ldweights(nc)
    _split_excess_waits(nc)
    # Populate .instr bytes for extended-inst InstISA subclasses
    # (InstPartitionBroadcast) — raw Bass skips this Bacc pass and the NEFF
    # compiler errors with "ISA wrong length" without it.
    lower_extended_insts(nc)
    return nc


def _prep_in_maps(in_features, q_proj, k_proj, v_proj, o_proj):
    # Host-side prep in numpy — np.asarray first so jax-array inputs don't
    # route the transpose/cast through a device backend.
    in_features = np.asarray(in_features)
    q_proj = np.asarray(q_proj)
    k_proj = np.asarray(k_proj)
    v_proj = np.asarray(v_proj)
    o_proj = np.asarray(o_proj)
    bf = ml_dtypes.bfloat16
    # mask variant r: [128, 512] keeping (1.0) where q >= k + 128r, else 0.
    k_idx = np.arange(128)[:, None]
    q_idx = np.arange(512)[None, :]
    masks = np.concatenate(
        [(q_idx >= k_idx + 128 * r) for r in range(NQB)], axis=1
    ).astype(bf)
    ones = np.ones((128, 1), bf)
    in_maps = []
    for c in range(N_CORES):
        b, g = divmod(c, 4)
        ms = slice(512 * g, 512 * (g + 1))
        in_maps.append(
            {
                "xt": in_features[b].T.astype(bf),
                "wqt": q_proj[ms, :].T.astype(bf),
                "wkt": k_proj[ms, :].T.astype(bf),
                "wvt": v_proj[ms, :].T.astype(bf),
                "wot": o_proj[:, ms].T.astype(bf),
                "masks": masks,
                "ones": ones,
            }
        )
    return in_maps


def _run(inputs, trace=False):
    nc = build_bass()
    in_maps = _prep_in_maps(**inputs)
    res = run_bass_kernel_spmd(nc, in_maps, list(range(N_CORES)), trace=trace)
    B = inputs["in_features"].shape[0]
    out = np.zeros((B, S, D), np.float32)
    for c in range(N_CORES):
        out[c // 4] += res.results[c]["out"]
    return out, res


def kernel(**inputs):
    out, _ = _run(inputs, trace=False)
    return out
